# revision 32
# baseline (speedup 1.0000x reference)
"""Multi-head attention (B=2, S=2048, D=768, H=12) on 8 trn2 NeuronCores.

Sharding: 24 (batch, head) pairs split 3-heads-per-core (core c -> batch
c//4, heads 3*(c%4)..+2).  Each core: Q/K/V projections for its heads,
attention, and a partial output projection against its 192-col slice of
w_o.  Host sums the per-batch partials (f32 from f16 partials).

Design (exp/ACT-paced weave; everything f16 on device, f32 only in PSUM):
 - host pre-casts inputs+weights to f16 and pre-transposes, halving input
   DMA and removing all on-device f32->f16 cast traffic; weights and
   output are shipped in partition-major packed layouts so every DMA
   moves >=1KB-contiguous per-partition lines
 - input DMAs are issued in consumption-order waves across the 3 issue
   queues; q cols 0-1023 are chunk-granular so the q-projection
   accumulates as data lands; the Act sequencer only carries early waves
 - kT/qT live in per-head MIRRORED tiles [128,S] (rows 0-63 == 64-127):
   the two 512-col score matmuls of a slab run as CONCURRENT PE
   row-tiles (tile_position (0,0)/(64,0), different PSUM banks),
   halving score time; mirror copies ride DVE in 4x f16 mode (~290ns)
 - V is projected token-major (lhsT = xvt chunk, rhs = wv [128,192])
   straight into vaug [128, chunk, head, 65] — no PE transposes; col 64
   is the ones column (softmax denominators fall out of the AV matmul)
 - scores computed transposed (S^T = K Q^T) per 128-key chunk into
   [128,1024] PSUM slabs (2 banks, double buffered); exp on the scalar
   engine; 1/sqrt(dk) folded into the Q-projection eviction
 - prologue eviction chain split DVE/ACT: only the h0-critical evicts
   +mirrors serialize on DVE before the first exp; h1/h2 ride ACT
 - softmax norm: evict xp to SBUF, den row to a base-0 tile, gpsimd
   partition_broadcast, DVE reciprocal_approx_fast, DVE multiply into
   xt01/xt2; the LAST slab instead broadcasts via a K=1 ones-matmul on
   the (then idle) PE into freed psX banks — shorter tail critical path
 - global 96-slot weave: per slot [exp(t) | scores-pair(t+1) | AV
   catch-up | fillers]; fillers ordered by DMA arrival (a filler whose
   data hasn't landed stalls every later PE instruction in FIFO);
   out-projection j0 fires at slots 44+, j1 at slots 78+ to keep the PE
   busy and HAM-warm through the end of slab 5
 - tail: j2 pair halves are emitted BEFORE the AV drain (they only need
   slabs 3/4 norms, so they overlap the exp-paced final AVs); j3 pair
   halves go to psX/psP banks freed by the last norm so the PE computes
   them during the norm-5 chain; h2 halves + evictions + out-DMAs drain
   in a pipelined ladder
"""

import sys

sys.path.insert(0, "/opt/trn_rl_repo")

import numpy as np

B, S, D = 2, 2048, 768
H, DK = 12, 64
HPC = 3          # heads per core
DP = HPC * DK    # 192 output dims per core per kind
NCORES = 8
DCH = D // 128   # 6 d-chunks
KC = S // 128    # 16 key chunks
QB = 512         # matmul N block
EB = 1024        # exp slab width
NEB = S // EB    # 2
SCALE = 1.0 / 8.0

_compiled = {}
import os as _os
_DBG = bool(_os.environ.get("KERNEL_DBG"))


def _build():
    import concourse.bass as bass
    import concourse.mybir as mybir
    import concourse.tile as tile
    from concourse import bacc

    f32 = mybir.dt.float32
    f16 = mybir.dt.float16
    MULT = mybir.AluOpType.mult
    EXP = mybir.ActivationFunctionType.Exp

    nc = bacc.Bacc("TRN2", target_bir_lowering=False, debug=False)

    xqt = nc.dram_tensor("xqt", [D, S], f16, kind="ExternalInput")
    xkt = nc.dram_tensor("xkt", [D, S], f16, kind="ExternalInput")
    xvt = nc.dram_tensor("xvt", [D, S], f16, kind="ExternalInput")
    # weights pre-packed [128, DCH*DP] partition-major so the DMA moves one
    # contiguous 2.3KB line per partition (the [D, DP] layout degenerates to
    # 384B descriptors and takes ~12us to land)
    wqt = nc.dram_tensor("wqt", [128, DCH * DP], f16, kind="ExternalInput")
    wkt = nc.dram_tensor("wkt", [128, DCH * DP], f16, kind="ExternalInput")
    wvt = nc.dram_tensor("wvt", [128, DCH * DP], f16, kind="ExternalInput")
    wot = nc.dram_tensor("wot", [DP, D], f16, kind="ExternalInput")
    outt = nc.dram_tensor("outt", [D, S], f16, kind="ExternalOutput")
    if _DBG:
        dbg_qT = nc.dram_tensor("dbg_qT", [128, S], f16, kind="ExternalOutput")
        dbg_kT = nc.dram_tensor("dbg_kT", [128, S], f16, kind="ExternalOutput")
        dbg_va = nc.dram_tensor("dbg_va", [128, KC * HPC * 65], f16, kind="ExternalOutput")
        dbg_pt = nc.dram_tensor("dbg_pt", [128, EB], f16, kind="ExternalOutput")
        dbg_xt = nc.dram_tensor("dbg_xt", [128, S], f16, kind="ExternalOutput")
        dbg_x2 = nc.dram_tensor("dbg_x2", [64, S], f16, kind="ExternalOutput")
        dbg_xc = nc.dram_tensor("dbg_xc", [65, QB], f32, kind="ExternalOutput")
        dbg_rd = nc.dram_tensor("dbg_rd", [1, QB], f32, kind="ExternalOutput")
        dbg_rdb = nc.dram_tensor("dbg_rdb", [64, QB], f32, kind="ExternalOutput")

    with tile.TileContext(nc) as tc:
        with (
            tc.tile_pool(name="res", bufs=1) as res,
            tc.tile_pool(name="pt", bufs=16) as pt_pool,
            tc.tile_pool(name="xc", bufs=4) as xc_pool,
            tc.tile_pool(name="ob", bufs=2) as ob_pool,
            tc.tile_pool(name="psS", bufs=2, space="PSUM") as psS,
            tc.tile_pool(name="psX", bufs=2, space="PSUM") as psX,
            tc.tile_pool(name="psP", bufs=2, space="PSUM") as psP,
        ):
            ring_state = [0]

            def ring3():
                ring_state[0] = (ring_state[0] + 1) % 3
                return (nc.sync, nc.scalar, nc.gpsimd)[ring_state[0]]

            ring2_state = [0]

            def ring2():
                # mid-kernel rings: keep the Act sequencer free for exps
                ring2_state[0] ^= 1
                return (nc.sync, nc.gpsimd)[ring2_state[0]]

            # ---- resident tiles ----
            wq_bf = res.tile([128, DCH, DP], f16, tag="wq")
            wk_bf = res.tile([128, DCH, DP], f16, tag="wk")
            wv_bf = res.tile([128, DCH, DP], f16, tag="wv")
            wo_pair = res.tile([128, D], f16, tag="wop")
            wo_h2 = res.tile([64, D], f16, tag="wo2")
            qt_bf = res.tile([128, DCH, S], f16, tag="qt")
            kt_bf = res.tile([128, DCH, S], f16, tag="kt")
            vt_bf = res.tile([128, DCH, S], f16, tag="vt")
            # kTm/qTm: per-head, MIRRORED across the two partition halves
            # (rows 0-63 == rows 64-127).  The two 512-col score matmuls of
            # a slab then run as CONCURRENT row-tiles — qh0 contracts in PE
            # rows 0-63, qh1 in rows 64-127 (tile_position auto-derives
            # from base_partition) — halving score time on the PE.
            kTm = [
                res.tile([128, S], f16, tag=f"kTm{h}", name=f"kTm{h}")
                for h in range(HPC)
            ]
            qTm = [
                res.tile([128, S], f16, tag=f"qTm{h}", name=f"qTm{h}")
                for h in range(HPC)
            ]
            # vaug [tokens, chunk, head, dims+1]: written DIRECTLY by the
            # token-major V projection (lhsT = xvt chunk), no PE transposes;
            # col 64 of each head slice is the softmax-denominator ones col
            vaug = res.tile([128, KC, HPC, 65], f16, tag="vaug", name="vaug")
            xt01 = res.tile([128, S], f16, tag="xt01")
            xt2 = res.tile([64, S], f16, tag="xt2")
            ones_row = res.tile([1, 64], f16, tag="ones_row", name="ones_row")
            nc.vector.memset(ones_row[:], 1.0)

            # ---- prologue DMAs in consumption-order waves (the 3 issue
            # queues share ~330GB/s of HBM; first slabs only need k/q cols
            # 0-1023, so later columns ride behind the pipeline) ----
            q1, q2, q3, q4 = (
                slice(0, QB),
                slice(QB, EB),
                slice(EB, EB + QB),
                slice(EB + QB, S),
            )
            lo, hi = slice(0, EB), slice(EB, S)
            # k-q1 chunks lead scalar+gpsimd with NOTHING ahead (the first
            # matmul's gate); weights ride sync; wk before wq (needed first)
            nc.sync.dma_start(wk_bf[:], wkt.rearrange("p (c o) -> p c o", c=DCH))
            for i in range(DCH):
                eng = nc.scalar if i % 2 == 0 else nc.gpsimd
                eng.dma_start(kt_bf[:, i, q1], xkt[128 * i : 128 * (i + 1), q1])
            nc.sync.dma_start(wq_bf[:], wqt.rearrange("p (c o) -> p c o", c=DCH))
            # waves 1b/1c: q cols 0-511 then 512-1023, chunk-granular and
            # striped over all 3 queues so the q-projection accumulates as
            # chunks land (scores-lo needs only j0; -hi adds j1)
            ring3_engs = (nc.sync, nc.scalar, nc.gpsimd)
            for jq in (q1, q2q := slice(QB, EB)):
                for i in range(DCH):
                    ring3_engs[i % 3].dma_start(
                        qt_bf[:, i, jq], xqt[128 * i : 128 * (i + 1), jq]
                    )
            # wave 2: k cols 512-1023 (k-proj j1 gates scores t=4-7); the
            # LAST scalar-queue issues — later ones would block the exps
            for g, eng in enumerate((nc.scalar, nc.gpsimd, nc.sync)):
                eng.dma_start(
                    kt_bf[:, 2 * g : 2 * g + 2, q2],
                    xkt[256 * g : 256 * (g + 1), q2].rearrange(
                        "(c p) s -> p c s", p=128
                    ),
                )
            # wave 3: k cols 1024-2047 (k-proj j2/j3 fillers run before the
            # v-proj fillers, so k-hi must land before v-lo; sync/gpsimd
            # only — the Act sequencer is busy with exps from ~14us)
            for g in range(3):
                ring2().dma_start(
                    kt_bf[:, 2 * g : 2 * g + 2, hi],
                    xkt[256 * g : 256 * (g + 1), hi].rearrange(
                        "(c p) s -> p c s", p=128
                    ),
                )
            nc.gpsimd.dma_start(wv_bf[:], wvt.rearrange("p (c o) -> p c o", c=DCH))
            for g in range(3):
                ring2().dma_start(
                    vt_bf[:, 2 * g : 2 * g + 2, lo],
                    xvt[256 * g : 256 * (g + 1), lo].rearrange(
                        "(c p) s -> p c s", p=128
                    ),
                )
            for g in range(3):
                ring2().dma_start(
                    vt_bf[:, 2 * g : 2 * g + 2, hi],
                    xvt[256 * g : 256 * (g + 1), hi].rearrange(
                        "(c p) s -> p c s", p=128
                    ),
                )
            nc.sync.dma_start(wo_pair[:], wot[0:128, :])
            nc.gpsimd.dma_start(wo_h2[:], wot[128:DP, :])
            for g in range(3):
                ring2().dma_start(
                    qt_bf[:, 2 * g : 2 * g + 2, hi],
                    xqt[256 * g : 256 * (g + 1), hi].rearrange(
                        "(c p) s -> p c s", p=128
                    ),
                )
            # ones columns for the softmax denominators (col 64 per head)
            nc.vector.memset(vaug[:, :, :, 64:65], 1.0)

            # ---- projection groups ----
            KINDS = {
                "q": (wq_bf, qt_bf, qTm),
                "k": (wk_bf, kt_bf, kTm),
            }

            def proj_group(kind, mt, j, ev_plan=None):
                """One M-tile (mt=0: heads01 pair M=128; mt=1: head2 M=64)
                of a 512-col projection block j.  Evictions write the row
                0-63 half of the per-head mirrored tile; the row 64-127
                mirror copy rides the (mostly idle) gpsimd engine.  ev_plan
                overrides (evict_engine, mirror_engine) per head — used in
                the prologue to split the eviction chain across DVE+ACT."""
                wbf, xbf, dstm = KINDS[kind]
                cols = slice(QB * j, QB * (j + 1))
                if mt == 0:
                    pp = psP.tile([128, QB], f32, tag="pp")
                    wsl = slice(0, 128)
                else:
                    pp = psP.tile([64, QB], f32, tag="pp")
                    wsl = slice(128, DP)
                for i in range(DCH):
                    nc.tensor.matmul(
                        pp[:],
                        wbf[:, i, wsl],
                        xbf[:, i, cols],
                        start=(i == 0),
                        stop=(i == DCH - 1),
                    )
                heads = (
                    ((0, pp[0:64, :]), (1, pp[64:128, :]))
                    if mt == 0
                    else ((2, pp[:]),)
                )
                for idx, (h, src) in enumerate(heads):
                    d = dstm[h]
                    # mirrors ALWAYS on DVE: f16 SBUF->SBUF packed runs in
                    # 4x mode (~287ns) there, vs ~1.9us on gpsimd
                    eeng = ev_plan[idx] if ev_plan else nc.vector
                    if kind == "q":
                        # fold the 1/sqrt(dk) softmax scale into the eviction
                        if eeng is nc.scalar:
                            nc.scalar.mul(d[0:64, cols], src, SCALE)
                        else:
                            eeng.tensor_scalar_mul(d[0:64, cols], src, SCALE)
                    else:
                        if eeng is nc.scalar:
                            nc.scalar.copy(d[0:64, cols], src)
                        else:
                            eeng.tensor_copy(d[0:64, cols], src)
                    nc.vector.tensor_copy(d[64:128, cols], d[0:64, cols])

            vchunk_done = [0]

            def count_vchunk(c):
                """Token-major V projection of one 128-token chunk straight
                into vaug (all 3 heads at once): out[tok, h*64+d] =
                sum_i xvt[i-chunk, tok] @ wv[i-chunk, h*64+d]."""
                def emit():
                    vp = psP.tile([128, HPC, 64], f32, tag="pp", name="vp")
                    for i in range(DCH):
                        nc.tensor.matmul(
                            vp[:],
                            vt_bf[:, i, 128 * c : 128 * (c + 1)],
                            wv_bf[:, i, :],
                            start=(i == 0),
                            stop=(i == DCH - 1),
                        )
                    nc.vector.tensor_copy(vaug[:, c, :, 0:64], vp[:])
                    vchunk_done[0] += 1
                return emit

            ob_tiles = {}

            def outproj_unit(j, m, tail=False):
                def emit():
                    if m == 0:
                        ob_tiles[j] = ob_pool.tile(
                            [128, DCH, QB], f16, tag="ob", name="ob"
                        )
                    op = psP.tile([128, QB], f32, tag="pp")
                    nc.tensor.matmul(
                        op[:],
                        wo_pair[:, 128 * m : 128 * (m + 1)],
                        xt01[:, QB * j : QB * (j + 1)],
                        start=True,
                        stop=False,
                    )
                    nc.tensor.matmul(
                        op[:],
                        wo_h2[:, 128 * m : 128 * (m + 1)],
                        xt2[:, QB * j : QB * (j + 1)],
                        start=False,
                        stop=True,
                    )
                    if tail and m % 2 == 0:
                        nc.scalar.copy(ob_tiles[j][:, m, :], op[:])
                    else:
                        nc.vector.tensor_copy(ob_tiles[j][:, m, :], op[:])
                    # DMA out per chunk-pair: drains early, halves sem count
                    if m % 2 == 1:
                        (ring3() if tail else ring2()).dma_start(
                            outt.rearrange("(c p) s -> p c s", p=128)[
                                :, m - 1 : m + 1, QB * j : QB * (j + 1)
                            ],
                            ob_tiles[j][:, m - 1 : m + 1, :],
                        )
                return emit

            # ---- slab machinery ----
            slabs = [(0, 0), (1, 0), (2, 0), (0, 1), (1, 1), (2, 1)]  # (h, e)

            sp_tiles = {}
            pt_tiles = {}
            xp_tiles = {}

            def emit_scores(si, t):
                # the two 512-col halves run CONCURRENTLY: qh0 as PE row-
                # tile (0,0) from the direct rows, qh1 as row-tile (64,0)
                # from the mirrored rows, draining into different PSUM banks
                h, e = slabs[si]
                sp = psS.tile([128, EB], f32, tag="sp")
                sp_tiles[(si, t)] = sp
                for half in range(2):
                    base = 64 * half
                    nc.tensor.matmul(
                        sp[:, QB * half : QB * (half + 1)],
                        kTm[h][base : base + 64, 128 * t : 128 * (t + 1)],
                        qTm[h][
                            base : base + 64,
                            EB * e + QB * half : EB * e + QB * (half + 1),
                        ],
                        start=True,
                        stop=True,
                    )

            def emit_exp(si, t):
                pt = pt_pool.tile([128, EB], f16, tag="pt")
                pt_tiles[(si, t)] = pt
                sp = sp_tiles.pop((si, t))
                if si == 0 and t == 0:
                    # split: the a-half exp starts before qb1's scores land
                    nc.scalar.activation(pt[:, 0:QB], sp[:, 0:QB], EXP)
                    nc.scalar.activation(pt[:, QB:EB], sp[:, QB:EB], EXP)
                else:
                    nc.scalar.activation(pt[:], sp[:], EXP)
                if _DBG and si == 0 and t == 0:
                    nc.sync.dma_start(dbg_pt[:], pt[:])

            def emit_norm(si, xpa, xpb):
                """Evict both [65,512] AV accumulators and normalize into xt.
                The two halves' ops are interleaved so the DVE never idles
                waiting for a gpsimd broadcast (partition_broadcast reads
                ABSOLUTE partition 0 on HW, hence the base-0 den copies)."""
                h, e = slabs[si]
                xca = xc_pool.tile([65, QB], f32, tag="xc", name="xca")
                xcb = xc_pool.tile([65, QB], f32, tag="xc", name="xcb")
                # last slab: ACT just issued its final exp, so it takes the
                # PSUM evictions — shortens the tail's norm critical path
                ev = nc.scalar.copy if si == len(slabs) - 1 else None
                (ev or nc.vector.tensor_copy)(xca[:], xpa[:])
                (ev or nc.vector.tensor_copy)(xcb[:], xpb[:])
                # tail slab: f16 den rows so they can feed the f16 ones-
                # matmul broadcast (dens are O(100..2000) — f16 is plenty)
                d0_dt = f16 if si == len(slabs) - 1 else f32
                d0a = xc_pool.tile([1, QB], d0_dt, tag="rd", name="d0a")
                d0b = xc_pool.tile([1, QB], d0_dt, tag="rd", name="d0b")
                nc.vector.tensor_copy(d0a[:], xca[64:65, :])
                nc.vector.tensor_copy(d0b[:], xcb[64:65, :])
                rba = xc_pool.tile([64, QB], f32, tag="rdb", name="rba")
                rbb = xc_pool.tile([64, QB], f32, tag="rdb", name="rbb")
                if si == len(slabs) - 1:
                    # last slab: its norm chain is the tail's critical path
                    # and the PE is idle — broadcast the denominator row via
                    # a K=1 ones-matmul (~0.4us) instead of the ~1us gpsimd
                    # partition_broadcast, into the just-freed psX banks
                    rbpa = psX.tile([64, QB], f32, tag="xp", name="rbpa")
                    rbpb = psX.tile([64, QB], f32, tag="xp", name="rbpb")
                    nc.tensor.matmul(
                        rbpa[:], ones_row[:], d0a[:], start=True, stop=True
                    )
                    nc.tensor.matmul(
                        rbpb[:], ones_row[:], d0b[:], start=True, stop=True
                    )
                    nc.vector.reciprocal_approx_fast(rba[:], rbpa[:])
                    nc.vector.reciprocal_approx_fast(rbb[:], rbpb[:])
                else:
                    nc.gpsimd.partition_broadcast(rba[:], d0a[:])
                    nc.gpsimd.partition_broadcast(rbb[:], d0b[:])
                    nc.vector.reciprocal_approx_fast(rba[:], rba[:])
                    nc.vector.reciprocal_approx_fast(rbb[:], rbb[:])
                for a, xc, rb in ((0, xca, rba), (1, xcb, rbb)):
                    cols = slice(EB * e + QB * a, EB * e + QB * (a + 1))
                    if h == 0:
                        dst = xt01[0:64, cols]
                    elif h == 1:
                        dst = xt01[64:128, cols]
                    else:
                        dst = xt2[:, cols]
                    nc.vector.tensor_tensor(dst, xc[0:64, :], rb[:], MULT)
                    normed[2 * e + a] += 1
                if _DBG and si == 0:
                    nc.sync.dma_start(dbg_xc[:], xca[:])
                    nc.sync.dma_start(dbg_rdb[:], rba[:])

            def emit_av(si, t):
                h, e = slabs[si]
                if si not in xp_tiles:
                    xp_tiles[si] = (
                        psX.tile([65, QB], f32, tag="xp", name="xpa"),
                        psX.tile([65, QB], f32, tag="xp", name="xpb"),
                    )
                xpa, xpb = xp_tiles[si]
                pt = pt_tiles[(si, t)]
                nc.tensor.matmul(
                    xpa[:], vaug[:, t, h, :], pt[:, 0:QB],
                    start=(t == 0), stop=(t == KC - 1),
                )
                nc.tensor.matmul(
                    xpb[:], vaug[:, t, h, :], pt[:, QB:EB],
                    start=(t == 0), stop=(t == KC - 1),
                )
                if t == KC - 1:
                    emit_norm(si, xpa, xpb)
                    del xp_tiles[si]
                    for u in range(KC):
                        del pt_tiles[(si, u)]

            # ---- filler queue (order encodes deadlines) ----
            normed = [0, 0, 0, 0]  # heads normed per 512-col block

            fillers = []
            # order encodes both deadlines AND DMA arrival: a filler whose
            # data hasn't landed stalls every later PE instruction (FIFO),
            # so each is placed at the slot where its data is in SBUF.
            # arrivals @330GB/s: k-q2 ~14us, q-lo ~19, k-hi ~23.5, v-lo
            # ~29, v-hi ~32, q-hi ~38; slot s executes at ~21+1.08*s us.
            fillers.append(lambda: proj_group("k", 0, 1))
            fillers.append(lambda: proj_group("q", 1, 0))
            fillers.append(lambda: proj_group("k", 0, 2))
            fillers.append(lambda: proj_group("q", 1, 1))
            fillers.append(lambda: proj_group("k", 0, 3))
            fillers.append(lambda: proj_group("k", 1, 1))
            for c in range(8):
                fillers.append(count_vchunk(c))
            # head-2 k projections (needed from slab 2 = slot 32)
            for j in range(2, 4):
                fillers.append(lambda j=j: proj_group("k", 1, j))
            for c in range(8, KC):
                fillers.append(count_vchunk(c))
            # q cols 1024-2047 (needed from slab 3 = slot 48)
            fillers.append(lambda: proj_group("q", 0, 2))
            fillers.append(lambda: proj_group("q", 0, 3))
            fillers.append(lambda: proj_group("q", 1, 2))
            fillers.append(lambda: proj_group("q", 1, 3))
            # early out-projection: j0 fires mid-weave; j1 is held back to
            # slots 76+ so the PE stays busy (and HAM-warm) through the end
            # of slab 5, where fillers otherwise run dry and the PE idles
            # at exp pace
            late_fillers = []
            for m in range(DCH):
                late_fillers.append(
                    (44 + 2 * m, lambda: normed[0] >= HPC, outproj_unit(0, m))
                )
            for m in range(DCH):
                late_fillers.append(
                    (78 + 3 * m, lambda: normed[1] >= HPC, outproj_unit(1, m))
                )

            # ---- prologue projections: enough for slab 0.  The first exp
            # chain needs ONLY the h0 evictions+mirrors — those ride DVE;
            # h1/h2 evictions go to ACT (idle until the first exp) and
            # their mirrors to gpsimd, so the critical DVE chain is 3
            # evicts + 3 mirrors instead of 7 evicts + 7 mirrors ----
            proj_group("k", 0, 0, ev_plan=[nc.vector, nc.scalar])
            proj_group("k", 1, 0, ev_plan=[nc.scalar])
            proj_group("q", 0, 0, ev_plan=[nc.vector, nc.scalar])
            proj_group("q", 0, 1, ev_plan=[nc.vector, nc.scalar])

            # ---- the 96-slot weave ----
            av_due = []   # (global_slot_emitted, si, t)
            av_ptr = [0]

            def pop_avs(s_now, budget):
                n = 0
                while n < budget and av_ptr[0] < len(av_due):
                    s_e, si, t = av_due[av_ptr[0]]
                    if s_e > s_now - 2:
                        break
                    if vchunk_done[0] <= t:
                        break
                    emit_av(si, t)
                    av_ptr[0] += 1
                    n += 1
                return n

            emit_scores(0, 0)
            for s in range(96):
                si, t = divmod(s, 16)
                emit_exp(si, t)
                av_due.append((s, si, t))
                if t < KC - 1:
                    emit_scores(si, t + 1)
                elif si < len(slabs) - 1:
                    emit_scores(si + 1, 0)
                npop = pop_avs(s, 5 if s >= 84 else (4 if s >= 16 else 3))
                nf = 2 if (npop == 0 or s < 16) else 1
                for _ in range(nf):
                    if not fillers:
                        break
                    head = fillers[0]
                    if isinstance(head, tuple):
                        gate, fn = head
                        if not gate():
                            break
                        fillers.pop(0)
                        fn()
                    else:
                        fillers.pop(0)()
                while late_fillers:
                    s_min, gate, fn = late_fillers[0]
                    if s >= s_min and gate():
                        late_fillers.pop(0)
                        fn()
                    else:
                        break

            # ---- tail: emit any leftover late fillers, then j2's PAIR
            # halves BEFORE the AV drain — they only depend on slabs 3/4
            # norms (done ~slot 83), so they jump ahead of the exp-paced
            # final AVs in the PE FIFO and keep the PE busy+HAM-warm.  The
            # h2 halves (which need slab 5's norm, i.e. the drained AVs)
            # follow the drain; j3 runs last reusing the freed banks. ----
            for u in late_fillers:
                u[2]()
            late_fillers = []

            def tail_pair(j):
                opS1 = psS.tile([128, EB], f32, tag="sp", name="opS1")
                opS2 = psS.tile([128, EB], f32, tag="sp", name="opS2")
                opP1 = psP.tile([128, QB], f32, tag="pp", name="opP1")
                opP2 = psP.tile([128, QB], f32, tag="pp", name="opP2")
                ops = [
                    opS1[:, 0:QB], opS1[:, QB:EB],
                    opS2[:, 0:QB], opS2[:, QB:EB],
                    opP1[:], opP2[:],
                ]
                obt = ob_pool.tile([128, DCH, QB], f16, tag="ob", name="ob")
                for m in range(DCH):
                    nc.tensor.matmul(
                        ops[m],
                        wo_pair[:, 128 * m : 128 * (m + 1)],
                        xt01[:, QB * j : QB * (j + 1)],
                        start=True,
                        stop=False,
                    )
                return ops, obt

            def tail_h2(j, ops, obt):
                for m in range(DCH):
                    nc.tensor.matmul(
                        ops[m],
                        wo_h2[:, 128 * m : 128 * (m + 1)],
                        xt2[:, QB * j : QB * (j + 1)],
                        start=False,
                        stop=True,
                    )
                    if m % 2 == 0:
                        nc.scalar.copy(obt[:, m, :], ops[m])
                    else:
                        nc.vector.tensor_copy(obt[:, m, :], ops[m])
                    if m % 2 == 1:
                        ring3().dma_start(
                            outt.rearrange("(c p) s -> p c s", p=128)[
                                :, m - 1 : m + 1, QB * j : QB * (j + 1)
                            ],
                            obt[:, m - 1 : m + 1, :],
                        )

            ops2, obt2 = tail_pair(2)
            while av_ptr[0] < len(av_due):
                pop_avs(10**9, 10**9)
            for u in fillers:
                u[1]() if isinstance(u, tuple) else u()
            fillers = []
            # j3 pair halves m0-3 go to the psX banks freed by the last
            # norm's evictions — they only need xt01, so the PE runs them
            # WHILE the norm-5 chain completes (and HAM stays warm)
            obt3 = ob_pool.tile([128, DCH, QB], f16, tag="ob", name="ob")
            ops3 = []

            def j3_pair(m, pool):
                op3 = psX.tile([128, QB], f32, tag="xp", name=f"op3_{m}") \
                    if pool is psX else \
                    psP.tile([128, QB], f32, tag="pp", name=f"op3_{m}")
                ops3.append(op3[:])
                nc.tensor.matmul(
                    op3[:],
                    wo_pair[:, 128 * m : 128 * (m + 1)],
                    xt01[:, QB * 3 : QB * 4],
                    start=True,
                    stop=False,
                )

            for m in range(4):
                j3_pair(m, psX)
            tail_h2(2, ops2, obt2)
            for m in (4, 5):
                j3_pair(m, psP)
            tail_h2(3, ops3, obt3)
            if _DBG:
                nc.sync.dma_start(dbg_qT[:], qTm[0][:])
                nc.sync.dma_start(dbg_kT[:], kTm[0][:])
                nc.sync.dma_start(
                    dbg_va[:], vaug.rearrange("p a b c -> p (a b c)")
                )
                nc.sync.dma_start(dbg_xt[:], xt01[:])
                nc.sync.dma_start(dbg_x2[:], xt2[:])

    nc.compile()
    return nc


def _get_nc():
    if "nc" not in _compiled:
        _compiled["nc"] = _build()
    return _compiled["nc"]


def _pack_w(w, cols):
    # [128, DCH*DP]: partition p, free = (chunk c, outdim o) of w[cols].T
    wt = w[cols, :].T.astype(np.float16)          # [D, DP]
    return np.ascontiguousarray(
        wt.reshape(DCH, 128, DP).transpose(1, 0, 2).reshape(128, DCH * DP)
    )


def _shard(q, k, v, w_q, w_k, w_v, w_o):
    f16 = np.float16
    in_maps = []
    for c in range(NCORES):
        b, g = divmod(c, NCORES // B)
        cols = slice(DP * g, DP * (g + 1))
        in_maps.append(
            {
                "xqt": np.ascontiguousarray(q[b].T.astype(f16)),
                "xkt": np.ascontiguousarray(k[b].T.astype(f16)),
                "xvt": np.ascontiguousarray(v[b].T.astype(f16)),
                "wqt": _pack_w(w_q, cols),
                "wkt": _pack_w(w_k, cols),
                "wvt": _pack_w(w_v, cols),
                "wot": np.ascontiguousarray(w_o[:, cols].T.astype(f16)),
            }
        )
    return in_maps


def kernel(q, k, v, w_q, w_k, w_v, w_o, _trace=False):
    from concourse.bass_utils import run_bass_kernel_spmd

    q = np.asarray(q, np.float32)
    k = np.asarray(k, np.float32)
    v = np.asarray(v, np.float32)
    w_q = np.asarray(w_q, np.float32)
    w_k = np.asarray(w_k, np.float32)
    w_v = np.asarray(w_v, np.float32)
    w_o = np.asarray(w_o, np.float32)

    nc = _get_nc()
    in_maps = _shard(q, k, v, w_q, w_k, w_v, w_o)
    res = run_bass_kernel_spmd(nc, in_maps, list(range(NCORES)), trace=_trace)
    out = np.zeros((B, S, D), np.float32)
    for c in range(NCORES):
        b = c // (NCORES // B)
        out[b] += res.results[c]["outt"].T.astype(np.float32)
    if _trace:
        return out, res
    return out

